# revision 1
# baseline (speedup 1.0000x reference)
"""Trainium2 Bass kernel for the 8-level butterfly layer.

Contract: kernel(**inputs) takes FULL unsharded numpy inputs
(in_data [512,4096], W_in [16,64], b_in [64], W_lvl [510,2,64,64],
b_lvl [510,64], Fea [256,64,16]) and returns the FULL output
(512, 4096, 1) float32.

Strategy: pure data parallelism over batch (64 rows per core, 8 cores),
butterfly filters replicated. Per core, each level is a set of K=128
contraction matmuls in bf16. The t-parity split needed by the next
level's pair concatenation is produced by the matmul itself via
column-tiled PE matmuls (even-t columns -> PSUM partitions 0:64, odd-t
-> 64:128), so every PSUM->SBUF relu+bias+cast runs on all 128
partitions with no partition shifts.

Activation layout per level L ("pair format"), one SBUF tensor
R_L [128, 8192] bf16: box c of level L occupies columns
[c*N, (c+1)*N), N = 2^(13-L); partition (s*64 + ch) holds channel ch of
position t with parity s; column within the box block is (t//2)*64 + b.
"""

import numpy as np
import ml_dtypes

import concourse.bass as bass
import concourse.mybir as mybir
import concourse.tile as tile
from concourse import bacc

NCORES = 8
B = 512
BC = B // NCORES  # 64 batch rows per core
NLVL = 8
C = 64
FIN = 16
FOUT = 16
KTOT = 256  # 2**NLVL
INS = 4096

BF16 = mybir.dt.bfloat16
F32 = mybir.dt.float32

_CACHE: dict = {}
_PHASES: list = []  # (phase_name, next_instruction_id_at_start) from last build


def _bf16(a: np.ndarray) -> np.ndarray:
    return np.ascontiguousarray(a.astype(np.float32)).astype(ml_dtypes.bfloat16)


def pack_shared(W_in, b_in, W_lvl, b_lvl, Fea) -> dict:
    """Host-side repacking of the replicated filter tensors."""
    W_in = np.asarray(W_in, np.float32)
    b_in = np.asarray(b_in, np.float32)
    W_lvl = np.asarray(W_lvl, np.float32)
    b_lvl = np.asarray(b_lvl, np.float32)
    Fea = np.asarray(Fea, np.float32)

    # Levels 1..7 weights: boxes are W_lvl[0:254] in level-major order.
    # [254, 2, 64, 64] -> [254, 128, 64] (row = s*64+c_in) -> [128, 254*64]
    wmain = W_lvl[0:254].reshape(254, 128, 64).transpose(1, 0, 2).reshape(128, 254 * 64)

    # Level 8 weights, pair-packed: pair p holds boxes 2p, 2p+1
    # (global idx 254+2p, 254+2p+1). [128, (p, j, m)] -> [128, 16384]
    w8 = W_lvl[254:510].reshape(128, 2, 128, 64).transpose(2, 0, 1, 3).reshape(128, 128 * 128)

    # Fea pair-packed blockdiag: [128 rows (j,ch), 128 pairs, 32]
    fea = np.zeros((128, 128, 32), np.float32)
    fea[0:64, :, 0:16] = Fea[0::2].transpose(1, 0, 2)
    fea[64:128, :, 16:32] = Fea[1::2].transpose(1, 0, 2)
    fea = fea.reshape(128, 128 * 32)

    # Biases (fp32): duplicated across partition halves for levels in..5.
    bin_h = np.concatenate([b_in, b_in]).reshape(128, 1)
    bd7 = np.concatenate([b_lvl[0:254], b_lvl[0:254]], axis=1).T.copy()  # [128, 254]

    # Input filter, K-stacked blockdiag: rows 0:16 feed even-k channels
    # (psum partitions 0:64), rows 16:32 odd-k (partitions 64:128).
    winp = np.zeros((32, 128), np.float32)
    winp[0:16, 0:64] = W_in
    winp[16:32, 64:128] = W_in

    # Levels 6-8 bias-matmul operands: psum[p, col] += lhsT[blk(col), p].
    # l6: 16 psums x 4 boxes x 128 cols; l7: 16 x 8 x 64; l8: 16 x 8 parents.
    b6 = b_lvl[62:126]          # level-6 boxes, [64, 64]
    bias6 = np.concatenate([b6, b6], axis=1).reshape(16, 4, 128)
    bias6 = bias6.transpose(1, 0, 2).reshape(4, 2048)
    b7 = b_lvl[126:254]
    bias7 = np.concatenate([b7, b7], axis=1).reshape(16, 8, 128)
    bias7 = bias7.transpose(1, 0, 2).reshape(8, 2048)
    b8 = b_lvl[254:510].reshape(128, 128)  # pair p rows (j,ch)
    bias8 = b8.reshape(16, 8, 128).transpose(1, 0, 2).reshape(8, 2048)
    ones4 = np.repeat(np.eye(4, dtype=np.float32), 128, axis=1)   # [4, 512]
    ones8 = np.repeat(np.eye(8, dtype=np.float32), 64, axis=1)    # [8, 512]

    return {
        "winp": _bf16(winp),
        "wmain": _bf16(wmain),
        "w8": _bf16(w8),
        "fea": _bf16(fea),
        "bin": np.ascontiguousarray(bin_h, np.float32),
        "bd7": np.ascontiguousarray(bd7, np.float32),
        "bias6": _bf16(bias6),
        "bias7": _bf16(bias7),
        "bias8": _bf16(bias8),
        "ones4": _bf16(ones4),
        "ones8": _bf16(ones8),
    }


def pack_x(x_shard: np.ndarray) -> np.ndarray:
    """[64, 4096] batch shard -> [32, 8192] bf16: row h*16+f holds
    x[b, (2k'+h)*16+f] at col k'*64+b (K-stacked even/odd k)."""
    xs = np.asarray(x_shard, np.float32).reshape(BC, 128, 2, FIN)
    return _bf16(xs.transpose(2, 3, 1, 0).reshape(32, 128 * BC))


def _build_module(loop_iters: int | None = None):
    """Build the bass module. loop_iters wraps the whole body in an
    on-device For_i loop (benchmarking only; graded path uses None)."""
    nc = bacc.Bacc("TRN2", target_bir_lowering=False, debug=False)

    xt = nc.dram_tensor("xt", [32, 128 * BC], BF16, kind="ExternalInput")
    winp = nc.dram_tensor("winp", [32, 128], BF16, kind="ExternalInput")
    wmain = nc.dram_tensor("wmain", [128, 254 * 64], BF16, kind="ExternalInput")
    w8 = nc.dram_tensor("w8", [128, 128 * 128], BF16, kind="ExternalInput")
    fea = nc.dram_tensor("fea", [128, 128 * 32], BF16, kind="ExternalInput")
    bin_t = nc.dram_tensor("bin", [128, 1], F32, kind="ExternalInput")
    bd7 = nc.dram_tensor("bd7", [128, 254], F32, kind="ExternalInput")
    bias6 = nc.dram_tensor("bias6", [4, 2048], BF16, kind="ExternalInput")
    bias7 = nc.dram_tensor("bias7", [8, 2048], BF16, kind="ExternalInput")
    bias8 = nc.dram_tensor("bias8", [8, 2048], BF16, kind="ExternalInput")
    ones4 = nc.dram_tensor("ones4", [4, 512], BF16, kind="ExternalInput")
    ones8 = nc.dram_tensor("ones8", [8, 512], BF16, kind="ExternalInput")
    out = nc.dram_tensor("out", [BC, KTOT * FOUT], F32, kind="ExternalOutput")

    relu = mybir.ActivationFunctionType.Relu
    evac_cnt = 0
    _PHASES.clear()

    def mark(name):
        _PHASES.append((name, int(nc.get_next_instruction_name().split("-")[1])))

    def evac(dst, src, bias_ap=None):
        nonlocal evac_cnt
        if evac_cnt % 2 == 0:
            nc.scalar.activation(dst, src, relu,
                                 bias=bias_ap if bias_ap is not None else 0.0)
        elif bias_ap is not None:
            nc.vector.tensor_scalar(
                dst, src, bias_ap, 0.0,
                op0=mybir.AluOpType.add, op1=mybir.AluOpType.max,
            )
        else:
            nc.vector.tensor_scalar(
                dst, src, 0.0, None, op0=mybir.AluOpType.max,
            )
        evac_cnt += 1

    import contextlib

    with tile.TileContext(nc) as tc:
        with (
            tc.tile_pool(name="wp", bufs=1) as wp,
            tc.tile_pool(name="xp", bufs=1) as xp,
            tc.tile_pool(name="rp", bufs=3) as rp,
            tc.tile_pool(name="op", bufs=3) as op,
            tc.tile_pool(name="ps", bufs=5, space="PSUM") as ps,
            tc.tile_pool(name="po", bufs=3, space="PSUM") as po,
            tc.For_i(0, loop_iters, 1) if loop_iters else contextlib.nullcontext(),
        ):
            x_sb = xp.tile([32, 128 * BC], BF16, tag="x")
            nc.sync.dma_start(x_sb[:, :], xt.ap())
            winp_sb = wp.tile([32, 128], BF16, tag="winp")
            nc.sync.dma_start(winp_sb[:, :], winp.ap())
            bin_sb = wp.tile([128, 1], F32, tag="bin")
            nc.sync.dma_start(bin_sb[:, :], bin_t.ap())
            bd7_sb = wp.tile([128, 254], F32, tag="bd7")
            nc.sync.dma_start(bd7_sb[:, :], bd7.ap())
            bias_sb = {}
            for nm, t, kdim in (("bias6", bias6, 4), ("bias7", bias7, 8),
                                ("bias8", bias8, 8)):
                bias_sb[nm] = wp.tile([kdim, 2048], BF16, tag=nm, name=nm)
                nc.sync.dma_start(bias_sb[nm][:, :], t.ap())
            ones4_sb = wp.tile([4, 512], BF16, tag="ones4")
            nc.sync.dma_start(ones4_sb[:, :], ones4.ap())
            ones8_sb = wp.tile([8, 512], BF16, tag="ones8")
            nc.sync.dma_start(ones8_sb[:, :], ones8.ap())

            wt = {}
            for lv in range(1, 8):
                nb = 2 ** lv  # boxes at this level
                wt[lv] = wp.tile([128, nb * 64], BF16, tag=f"wt{lv}", name=f"wt{lv}")
                c0 = (2 ** lv - 2) * 64
                nc.sync.dma_start(wt[lv][:, :], wmain.ap()[:, c0:c0 + nb * 64])
            w8_sb = []
            for h in range(2):
                t = wp.tile([128, 8192], BF16, tag=f"w8{h}", name=f"w8{h}")
                nc.sync.dma_start(t[:, :], w8.ap()[:, h * 8192:(h + 1) * 8192])
                w8_sb.append(t)
            fea_sb = wp.tile([128, 128 * 32], BF16, tag="fea")
            nc.sync.dma_start(fea_sb[:, :], fea.ap())

            # ---- input interpolation: K-stacked [32,128] blockdiag W_in
            mark("input")
            R = rp.tile([128, 8192], BF16, tag="R")
            for j in range(16):
                pc = ps.tile([128, 512], F32, tag="ps")
                nc.tensor.matmul(
                    pc[:, :], lhsT=winp_sb[:, :],
                    rhs=x_sb[:, j * 512:(j + 1) * 512],
                    start=True, stop=True,
                )
                evac(R[:, j * 512:(j + 1) * 512], pc[:, :], bin_sb[:, :])

            # ---- butterfly levels 1..5 (per-box psum chunks, bias in evac)
            for lv in range(1, 6):
                mark(f"l{lv}")
                P = 2 ** (lv - 1)            # parent boxes at level lv-1
                Np = 2 ** (14 - lv)          # parent block columns
                Ncb = Np // 2                # child block columns
                S = min(512, Ncb)            # psum chunk columns
                TU = S // 64                 # t-pair units per chunk
                Rn = rp.tile([128, 8192], BF16, tag="R")
                for p in range(P):
                    pv = Rp_view = R[:, p * Np:(p + 1) * Np].rearrange(
                        "a (t2 two b) -> a t2 two b", two=2, b=64)
                    for cl in range(2):
                        box = 2 * p + cl
                        lhsT = wt[lv][:, box * 64:(box + 1) * 64]
                        for j in range(Ncb // S):
                            pc = ps.tile([128, 512], F32, tag="ps")
                            for q in range(2):
                                rhs = pv[:, j * TU:(j + 1) * TU, q, :]
                                nc.tensor.matmul(
                                    pc[64 * q:64 * (q + 1), 0:S], lhsT=lhsT, rhs=rhs,
                                    start=True, stop=True, tile_position=(0, 64 * q),
                                )
                            bc = 2 ** lv - 2 + box
                            evac(
                                Rn[:, box * Ncb + j * S: box * Ncb + (j + 1) * S],
                                pc[:, 0:S], bd7_sb[:, bc:bc + 1],
                            )
                R = Rn

            # ---- levels 6-7: bias via K=4/8 ones-pattern matmul, merged
            # 512-col psums + single relu-only evac per psum.
            for lv, nbx, ones_sb in ((6, 4, ones4_sb), (7, 8, ones8_sb)):
                mark(f"l{lv}")
                Np = 2 ** (14 - lv)
                Ncb = Np // 2
                bsb = bias_sb[f"bias{lv}"]
                Rn = rp.tile([128, 8192], BF16, tag="R")
                for i in range(16):
                    pc = ps.tile([128, 512], F32, tag="ps")
                    nc.tensor.matmul(
                        pc[:, :], lhsT=bsb[:, i * 128:(i + 1) * 128],
                        rhs=ones_sb[:, :], start=True, stop=False,
                    )
                    for bl in range(nbx):
                        box = nbx * i + bl
                        p = box // 2
                        pv = R[:, p * Np:(p + 1) * Np].rearrange(
                            "a (t2 two b) -> a t2 two b", two=2, b=64)
                        lhsT = wt[lv][:, box * 64:(box + 1) * 64]
                        for q in range(2):
                            nc.tensor.matmul(
                                pc[64 * q:64 * (q + 1),
                                   bl * Ncb:(bl + 1) * Ncb],
                                lhsT=lhsT, rhs=pv[:, :, q, :],
                                start=False, stop=True,
                                tile_position=(0, 64 * q),
                            )
                    evac(Rn[:, i * 512:(i + 1) * 512], pc[:, :])
                R = Rn

            # ---- level 8 (pair-packed, bias matmul, merged psums) with the
            # output stage interleaved: out batch bt consumes V8 groups
            # 16bt..16bt+15 = l8 psums i=2bt, 2bt+1.
            mark("l8")
            Rn = rp.tile([128, 8192], BF16, tag="R")
            for i in range(16):
                pc = ps.tile([128, 512], F32, tag="ps")
                nc.tensor.matmul(
                    pc[:, :], lhsT=bias_sb["bias8"][:, i * 128:(i + 1) * 128],
                    rhs=ones8_sb[:, :], start=True, stop=False,
                )
                for pl in range(8):
                    p = 8 * i + pl
                    lhsT = w8_sb[p // 64][:, (p % 64) * 128:(p % 64) * 128 + 128]
                    nc.tensor.matmul(
                        pc[:, pl * 64:(pl + 1) * 64], lhsT=lhsT,
                        rhs=R[:, p * 64:(p + 1) * 64],
                        start=False, stop=True,
                    )
                evac(Rn[:, i * 512:(i + 1) * 512], pc[:, :])
                if i % 2 == 1:
                    # out batch for V8 groups of psums i-1, i
                    bt = i // 2
                    pco = po.tile([BC, 512], F32, tag="po")
                    for g in range(16):
                        p = bt * 16 + g
                        nc.tensor.matmul(
                            pco[:, g * 32:(g + 1) * 32],
                            lhsT=Rn[:, p * 64:(p + 1) * 64],
                            rhs=fea_sb[:, p * 32:(p + 1) * 32],
                            start=True, stop=True,
                        )
                    o_sb = op.tile([BC, 512], F32, tag="os")
                    if bt % 2 == 0:
                        nc.scalar.copy(o_sb[:, :], pco[:, :])
                    else:
                        nc.vector.tensor_copy(o_sb[:, :], pco[:, :])
                    nc.sync.dma_start(out.ap()[:, bt * 512:(bt + 1) * 512],
                                      o_sb[:, :])
            R = Rn

    nc.compile()
    return nc


def _make_runner(nc):
    """Cached jitted SPMD runner over the 8 cores (mirrors
    bass2jax.run_bass_via_pjrt, but reusable across calls and without
    donation — this kernel writes every output element)."""
    import jax

    from concourse.bass2jax import (
        _bass_exec_p,
        install_neuronx_cc_hook,
        partition_id_tensor,
    )
    from jax.experimental.shard_map import shard_map
    from jax.sharding import Mesh, PartitionSpec

    install_neuronx_cc_hook()

    partition_name = nc.partition_id_tensor.name if nc.partition_id_tensor else None
    in_names: list[str] = []
    out_names: list[str] = []
    out_avals = []
    zero_outs: list[np.ndarray] = []
    for alloc in nc.m.functions[0].allocations:
        if not isinstance(alloc, mybir.MemoryLocationSet):
            continue
        name = alloc.memorylocations[0].name
        if alloc.kind == "ExternalInput":
            if name != partition_name:
                in_names.append(name)
        elif alloc.kind == "ExternalOutput":
            shape = tuple(alloc.tensor_shape)
            dtype = mybir.dt.np(alloc.dtype)
            out_names.append(name)
            out_avals.append(jax.core.ShapedArray(shape, dtype))
            zero_outs.append(np.zeros(shape, dtype))
    n_params = len(in_names)
    all_names = in_names + out_names
    if partition_name is not None:
        all_names = all_names + [partition_name]

    def _body(*args):
        operands = list(args)
        if partition_name is not None:
            operands.append(partition_id_tensor())
        outs = _bass_exec_p.bind(
            *operands,
            out_avals=tuple(out_avals),
            in_names=tuple(all_names),
            out_names=tuple(out_names),
            lowering_input_output_aliases=(),
            sim_require_finite=True,
            sim_require_nnan=True,
            nc=nc,
        )
        return tuple(outs)

    devices = jax.devices()[:NCORES]
    mesh = Mesh(np.asarray(devices), ("core",))
    n_all = n_params + len(out_names)
    sharded = jax.jit(
        shard_map(
            _body, mesh=mesh,
            in_specs=(PartitionSpec("core"),) * n_all,
            out_specs=(PartitionSpec("core"),) * len(out_names),
            check_rep=False,
        ),
        keep_unused=True,
    )
    return {
        "fn": sharded,
        "in_names": in_names,
        "out_names": out_names,
        "out_avals": out_avals,
        "zero_outs": zero_outs,
    }


def _runner():
    if "nc" not in _CACHE:
        _CACHE["nc"] = _build_module()
    if "runner" not in _CACHE:
        _CACHE["runner"] = _make_runner(_CACHE["nc"])
    return _CACHE["runner"]


def _concat_args(in_maps):
    r = _runner()
    args = [
        np.concatenate([np.asarray(m[name]) for m in in_maps], axis=0)
        for name in r["in_names"]
    ]
    args += [
        np.zeros((NCORES * z.shape[0], *z.shape[1:]), z.dtype) for z in r["zero_outs"]
    ]
    return args


def kernel(**inputs) -> np.ndarray:
    r = _runner()
    shared = pack_shared(
        inputs["W_in"], inputs["b_in"], inputs["W_lvl"], inputs["b_lvl"], inputs["Fea"]
    )
    in_data = np.asarray(inputs["in_data"], np.float32)
    in_maps = []
    for c in range(NCORES):
        m = dict(shared)
        m["xt"] = pack_x(in_data[c * BC:(c + 1) * BC])
        in_maps.append(m)

    out_arrs = r["fn"](*_concat_args(in_maps))
    out = np.asarray(out_arrs[r["out_names"].index("out")])
    return out.reshape(B, KTOT * FOUT, 1).astype(np.float32)



# revision 5
# speedup vs baseline: 1.2991x; 1.2991x over previous
"""Trainium2 Bass kernel for the 8-level butterfly layer.

Contract: kernel(**inputs) takes FULL unsharded numpy inputs
(in_data [512,4096], W_in [16,64], b_in [64], W_lvl [510,2,64,64],
b_lvl [510,64], Fea [256,64,16]) and returns the FULL output
(512, 4096, 1) float32.

Strategy: pure data parallelism over batch (64 rows per core, 8 cores),
butterfly filters replicated. Per core, each level is a set of
contraction matmuls in bf16. The t-parity split needed by the next
level's pair concatenation is produced by the matmul itself via
column-tiled PE matmuls (even-t columns -> PSUM partitions 0:64, odd-t
-> 64:128) for levels 1-6; level 7 uses paired-children matmuls
(lhsT [128,128] = both children of one parent) into a (child, ch)
scratch layout, fixed up by 4 SBUF->SBUF DVE copies.

Activation layout per level L ("pair format"), one SBUF tensor
R_L [128, 8192] bf16: box c of level L occupies columns
[c*N, (c+1)*N), N = 2^(13-L); partition (s*64 + ch) holds channel ch of
position t with parity s; column within the box block is (t//2)*64 + b.

Bias at levels 6-8 is accumulated into PSUM by a "fast bias matmul":
rhs = a 128-partition block-diagonal ones tensor (streams at full PE
rate; narrow-partition rhs streams ~3x slower), lhsT = a [128,128]
per-psum bias pattern with one nonzero injection row per column block.
"""

import numpy as np
import ml_dtypes

import concourse.bass as bass
import concourse.mybir as mybir
import concourse.tile as tile
from concourse import bacc

NCORES = 8
B = 512
BC = B // NCORES  # 64 batch rows per core
NLVL = 8
C = 64
FIN = 16
FOUT = 16
KTOT = 256  # 2**NLVL
INS = 4096

BF16 = mybir.dt.bfloat16
F32 = mybir.dt.float32

_CACHE: dict = {}
_PHASES: list = []  # (phase_name, next_instruction_id_at_start) from last build


def _bf16(a: np.ndarray) -> np.ndarray:
    return np.ascontiguousarray(a.astype(np.float32)).astype(ml_dtypes.bfloat16)


def pack_shared(W_in, b_in, W_lvl, b_lvl, Fea) -> dict:
    """Host-side repacking of the replicated filter tensors."""
    W_in = np.asarray(W_in, np.float32)
    b_in = np.asarray(b_in, np.float32)
    W_lvl = np.asarray(W_lvl, np.float32)
    b_lvl = np.asarray(b_lvl, np.float32)
    Fea = np.asarray(Fea, np.float32)

    # Levels 1..6 weights: boxes W_lvl[0:126] in level-major order.
    # [126, 2, 64, 64] -> [126, 128, 64] (row = s*64+c_in) -> [128, 126*64]
    wmain = W_lvl[0:126].reshape(126, 128, 64).transpose(1, 0, 2).reshape(128, 126 * 64)

    # Level 7 weights, children-paired: parent pa (l6 box 0..63) ->
    # lhsT [128 rows (s,c), 128 cols (cl,ch)] = [W(2pa) | W(2pa+1)].
    w7 = W_lvl[126:254].reshape(64, 2, 2, 64, 64)  # [pa, cl, s, c, d]
    wpair7 = w7.transpose(2, 3, 0, 1, 4).reshape(128, 64 * 128)

    # Level 8 weights, pair-packed: pair p holds boxes 2p, 2p+1
    # (global idx 254+2p, 254+2p+1). [128, (p, j, m)] -> [128, 16384]
    w8 = W_lvl[254:510].reshape(128, 2, 128, 64).transpose(2, 0, 1, 3).reshape(128, 128 * 128)

    # Fea pair-packed blockdiag: [128 rows (j,ch), 128 pairs, 32]
    fea = np.zeros((128, 128, 32), np.float32)
    fea[0:64, :, 0:16] = Fea[0::2].transpose(1, 0, 2)
    fea[64:128, :, 16:32] = Fea[1::2].transpose(1, 0, 2)
    fea = fea.reshape(128, 128 * 32)

    # Biases (fp32): duplicated across partition halves for levels in..5.
    bin_h = np.concatenate([b_in, b_in]).reshape(128, 1)
    bd7 = np.concatenate([b_lvl[0:62], b_lvl[0:62]], axis=1).T.copy()  # [128, 62]

    # Input filter, K-stacked blockdiag: rows 0:16 feed even-k channels
    # (psum partitions 0:64), rows 16:32 odd-k (partitions 64:128).
    winp = np.zeros((32, 128), np.float32)
    winp[0:16, 0:64] = W_in
    winp[16:32, 64:128] = W_in

    # Fast-bias operands for levels 6-8: ones tensors with one nonzero
    # injection row per column block, bias lhsT [128, 128] per psum.
    ones4b = np.zeros((128, 512), np.float32)
    for bl in range(4):
        ones4b[32 * bl, 128 * bl:128 * (bl + 1)] = 1.0
    ones8b = np.zeros((128, 512), np.float32)
    for pl in range(8):
        ones8b[16 * pl, 64 * pl:64 * (pl + 1)] = 1.0

    # l6 psum i: [128 (q,ch), 4 boxes (bl) x 128]; box = 4i+bl, bias row
    # b_lvl[62+box] replicated over q.
    b6 = b_lvl[62:126]
    biasT6 = np.zeros((128, 16, 2, 64), np.float32)
    for bl in range(4):
        biasT6[32 * bl, :, 0, :] = b6[bl::4]
        biasT6[32 * bl, :, 1, :] = b6[bl::4]
    biasT6 = biasT6.reshape(128, 2048)

    # l7 (paired) psum i: [128 (cl,ch), 4 parents (pl) x 128];
    # parent = 4i+pl, box = 2*parent+cl, bias b_lvl[126+box].
    b7 = b_lvl[126:254]
    biasT7 = np.zeros((128, 16, 2, 64), np.float32)
    for pl in range(4):
        for cl in range(2):
            biasT7[32 * pl, :, cl, :] = b7[2 * pl + cl::8]
    biasT7 = biasT7.reshape(128, 2048)

    # l8 psum i: [128 (j,ch), 8 pairs (pl) x 64]; pair = 8i+pl,
    # box = 254+2*pair+j.
    b8 = b_lvl[254:510]
    biasT8 = np.zeros((128, 16, 2, 64), np.float32)
    for pl in range(8):
        for j in range(2):
            biasT8[16 * pl, :, j, :] = b8[2 * pl + j::16]
    biasT8 = biasT8.reshape(128, 2048)

    return {
        "winp": _bf16(winp),
        "wmain": _bf16(wmain),
        "wpair7": _bf16(wpair7),
        "w8": _bf16(w8),
        "fea": _bf16(fea),
        "bin": np.ascontiguousarray(bin_h, np.float32),
        "bd7": np.ascontiguousarray(bd7, np.float32),
        "ones4b": _bf16(ones4b),
        "ones8b": _bf16(ones8b),
        "biasT6": _bf16(biasT6),
        "biasT7": _bf16(biasT7),
        "biasT8": _bf16(biasT8),
    }


def pack_x(x_shard: np.ndarray) -> np.ndarray:
    """[64, 4096] batch shard -> [32, 8192] bf16: row h*16+f holds
    x[b, (2k'+h)*16+f] at col k'*64+b (K-stacked even/odd k)."""
    xs = np.asarray(x_shard, np.float32).reshape(BC, 128, 2, FIN)
    return _bf16(xs.transpose(2, 3, 1, 0).reshape(32, 128 * BC))


def _build_module(loop_iters: int | None = None):
    """Build the bass module. loop_iters wraps the whole body in an
    on-device For_i loop (benchmarking only; graded path uses None)."""
    nc = bacc.Bacc("TRN2", target_bir_lowering=False, debug=False)

    xt = nc.dram_tensor("xt", [32, 128 * BC], BF16, kind="ExternalInput")
    winp = nc.dram_tensor("winp", [32, 128], BF16, kind="ExternalInput")
    wmain = nc.dram_tensor("wmain", [128, 126 * 64], BF16, kind="ExternalInput")
    wpair7 = nc.dram_tensor("wpair7", [128, 64 * 128], BF16, kind="ExternalInput")
    w8 = nc.dram_tensor("w8", [128, 128 * 128], BF16, kind="ExternalInput")
    fea = nc.dram_tensor("fea", [128, 128 * 32], BF16, kind="ExternalInput")
    bin_t = nc.dram_tensor("bin", [128, 1], F32, kind="ExternalInput")
    bd7 = nc.dram_tensor("bd7", [128, 62], F32, kind="ExternalInput")
    ones4b = nc.dram_tensor("ones4b", [128, 512], BF16, kind="ExternalInput")
    ones8b = nc.dram_tensor("ones8b", [128, 512], BF16, kind="ExternalInput")
    biasT6 = nc.dram_tensor("biasT6", [128, 2048], BF16, kind="ExternalInput")
    biasT7 = nc.dram_tensor("biasT7", [128, 2048], BF16, kind="ExternalInput")
    biasT8 = nc.dram_tensor("biasT8", [128, 2048], BF16, kind="ExternalInput")
    out = nc.dram_tensor("out", [BC, KTOT * FOUT], F32, kind="ExternalOutput")

    relu = mybir.ActivationFunctionType.Relu
    _PHASES.clear()

    def mark(name):
        _PHASES.append((name, int(nc.get_next_instruction_name().split("-")[1])))

    # Engine rotation for PSUM->SBUF relu(+bias) evacs. GPSIMD cannot
    # access PSUM (verifier-enforced), so only Act/DVE evacuate.
    evac_rot = {"i": 0}
    ROT = ("s", "v")

    def evac(dst, src, bias_ap=None, engines=ROT):
        e = engines[evac_rot["i"] % len(engines)]
        evac_rot["i"] += 1
        if e == "s":
            nc.scalar.activation(dst, src, relu,
                                 bias=bias_ap if bias_ap is not None else 0.0)
        elif e == "v":
            if bias_ap is not None:
                nc.vector.tensor_scalar(
                    dst, src, bias_ap, 0.0,
                    op0=mybir.AluOpType.add, op1=mybir.AluOpType.max,
                )
            else:
                nc.vector.tensor_scalar(
                    dst, src, 0.0, None, op0=mybir.AluOpType.max,
                )
        else:
            if bias_ap is not None:
                nc.gpsimd.tensor_scalar(
                    dst, src, bias_ap, 0.0,
                    op0=mybir.AluOpType.add, op1=mybir.AluOpType.max,
                )
            else:
                nc.gpsimd.tensor_scalar(
                    dst, src, 0.0, None, op0=mybir.AluOpType.max,
                )

    import contextlib

    with tile.TileContext(nc) as tc:
        with (
            tc.tile_pool(name="wp", bufs=1) as wp,
            tc.tile_pool(name="xp", bufs=1) as xp,
            tc.tile_pool(name="rp", bufs=3) as rp,
            tc.tile_pool(name="sp", bufs=1) as sp,
            tc.tile_pool(name="op", bufs=3) as op,
            tc.tile_pool(name="ps", bufs=5, space="PSUM") as ps,
            tc.tile_pool(name="po", bufs=3, space="PSUM") as po,
            tc.For_i(0, loop_iters, 1) if loop_iters else contextlib.nullcontext(),
        ):
            x_sb = xp.tile([32, 128 * BC], BF16, tag="x")
            nc.sync.dma_start(x_sb[:, :], xt.ap())
            winp_sb = wp.tile([32, 128], BF16, tag="winp")
            nc.sync.dma_start(winp_sb[:, :], winp.ap())
            bin_sb = wp.tile([128, 1], F32, tag="bin")
            nc.sync.dma_start(bin_sb[:, :], bin_t.ap())
            bd7_sb = wp.tile([128, 62], F32, tag="bd7")
            nc.sync.dma_start(bd7_sb[:, :], bd7.ap())
            fast_sb = {}
            for nm, t, w in (("ones4b", ones4b, 512), ("ones8b", ones8b, 512),
                             ("biasT6", biasT6, 2048), ("biasT7", biasT7, 2048),
                             ("biasT8", biasT8, 2048)):
                fast_sb[nm] = wp.tile([128, w], BF16, tag=nm, name=nm)
                nc.sync.dma_start(fast_sb[nm][:, :], t.ap())

            wt = {}
            for lv in range(1, 7):
                nb = 2 ** lv  # boxes at this level
                wt[lv] = wp.tile([128, nb * 64], BF16, tag=f"wt{lv}", name=f"wt{lv}")
                c0 = (2 ** lv - 2) * 64
                nc.sync.dma_start(wt[lv][:, :], wmain.ap()[:, c0:c0 + nb * 64])
            wp7_sb = wp.tile([128, 64 * 128], BF16, tag="wp7")
            nc.sync.dma_start(wp7_sb[:, :], wpair7.ap())
            w8_sb = []
            for h in range(2):
                t = wp.tile([128, 8192], BF16, tag=f"w8{h}", name=f"w8{h}")
                nc.sync.dma_start(t[:, :], w8.ap()[:, h * 8192:(h + 1) * 8192])
                w8_sb.append(t)
            fea_sb = wp.tile([128, 128 * 32], BF16, tag="fea")
            nc.sync.dma_start(fea_sb[:, :], fea.ap())

            # ---- input interpolation: K-stacked [32,128] blockdiag W_in
            mark("input")
            R = rp.tile([128, 8192], BF16, tag="R")
            for j in range(16):
                pc = ps.tile([128, 512], F32, tag="ps")
                nc.tensor.matmul(
                    pc[:, :], lhsT=winp_sb[:, :],
                    rhs=x_sb[:, j * 512:(j + 1) * 512],
                    start=True, stop=True,
                )
                evac(R[:, j * 512:(j + 1) * 512], pc[:, :], bin_sb[:, :])

            # ---- butterfly levels 1..5 (per-box psum chunks, bias in evac)
            for lv in range(1, 6):
                mark(f"l{lv}")
                P = 2 ** (lv - 1)            # parent boxes at level lv-1
                Np = 2 ** (14 - lv)          # parent block columns
                Ncb = Np // 2                # child block columns
                S = min(512, Ncb)            # psum chunk columns
                TU = S // 64                 # t-pair units per chunk
                Rn = rp.tile([128, 8192], BF16, tag="R")
                for p in range(P):
                    pv = R[:, p * Np:(p + 1) * Np].rearrange(
                        "a (t2 two b) -> a t2 two b", two=2, b=64)
                    for cl in range(2):
                        box = 2 * p + cl
                        lhsT = wt[lv][:, box * 64:(box + 1) * 64]
                        for j in range(Ncb // S):
                            pc = ps.tile([128, 512], F32, tag="ps")
                            for q in range(2):
                                rhs = pv[:, j * TU:(j + 1) * TU, q, :]
                                nc.tensor.matmul(
                                    pc[64 * q:64 * (q + 1), 0:S], lhsT=lhsT, rhs=rhs,
                                    start=True, stop=True, tile_position=(0, 64 * q),
                                )
                            bc = 2 ** lv - 2 + box
                            evac(
                                Rn[:, box * Ncb + j * S: box * Ncb + (j + 1) * S],
                                pc[:, 0:S], bd7_sb[:, bc:bc + 1],
                            )
                R = Rn

            # ---- level 6: merged 512-col psums, fast bias matmul
            # (128-partition blockdiag ones rhs), relu-only evac.
            mark("l6")
            Np = 2 ** 8          # parent (l5) block columns = 256
            Ncb = 128            # child block columns
            Rn = rp.tile([128, 8192], BF16, tag="R")
            for i in range(16):
                pc = ps.tile([128, 512], F32, tag="ps")
                nc.tensor.matmul(
                    pc[:, :], lhsT=fast_sb["biasT6"][:, i * 128:(i + 1) * 128],
                    rhs=fast_sb["ones4b"][:, :], start=True, stop=False,
                )
                for bl in range(4):
                    box = 4 * i + bl
                    p = box // 2
                    pv = R[:, p * Np:(p + 1) * Np].rearrange(
                        "a (t2 two b) -> a t2 two b", two=2, b=64)
                    lhsT = wt[6][:, box * 64:(box + 1) * 64]
                    for q in range(2):
                        nc.tensor.matmul(
                            pc[64 * q:64 * (q + 1), bl * Ncb:(bl + 1) * Ncb],
                            lhsT=lhsT, rhs=pv[:, :, q, :],
                            start=False, stop=True,
                            tile_position=(0, 64 * q),
                        )
                evac(Rn[:, i * 512:(i + 1) * 512], pc[:, :])
            R = Rn

            # ---- level 7: children-paired matmuls into (cl, ch) scratch,
            # fast bias matmul, then 8 DVE SBUF->SBUF shuffles to the
            # (t-parity, ch) layout level 8 expects.
            mark("l7")
            S7 = sp.tile([128, 8192], BF16, tag="S7")
            step1 = ("s", "s", "v", "s")
            for i in range(16):
                pc = ps.tile([128, 512], F32, tag="ps")
                nc.tensor.matmul(
                    pc[:, :], lhsT=fast_sb["biasT7"][:, i * 128:(i + 1) * 128],
                    rhs=fast_sb["ones4b"][:, :], start=True, stop=False,
                )
                for pl in range(4):
                    parent = 4 * i + pl
                    nc.tensor.matmul(
                        pc[:, pl * 128:(pl + 1) * 128],
                        lhsT=wp7_sb[:, parent * 128:(parent + 1) * 128],
                        rhs=R[:, parent * 128:(parent + 1) * 128],
                        start=False, stop=True,
                    )
                evac(S7[:, i * 512:(i + 1) * 512], pc[:, :], None, step1)
            Rn = rp.tile([128, 8192], BF16, tag="R")
            # S7 col = parent*128 + t7*64 + b; R7 col = (2*parent+cl)*64 + b
            # = parent*128 + cl*64 + b.  dst[par-half][.., cl, b] =
            # src[cl-half][.., t7=par, b].
            s7v = S7[:, :].rearrange("a (p t b) -> a p t b", t=2, b=64)
            r7v = Rn[:, :].rearrange("a (p t b) -> a p t b", t=2, b=64)
            for h in range(2):
                for cl in range(2):
                    for par in range(2):
                        nc.vector.tensor_copy(
                            r7v[64 * par:64 * (par + 1), h * 32:(h + 1) * 32, cl, :],
                            s7v[64 * cl:64 * (cl + 1), h * 32:(h + 1) * 32, par, :],
                        )
            R = Rn

            # ---- level 8 (pair-packed, fast bias matmul, merged psums)
            # with the output stage interleaved: out batch bt consumes V8
            # pairs 16bt..16bt+15 = l8 psums i=2bt, 2bt+1.
            mark("l8")
            Rn = rp.tile([128, 8192], BF16, tag="R")
            l8rot = ("s", "v")
            for i in range(16):
                pc = ps.tile([128, 512], F32, tag="ps")
                nc.tensor.matmul(
                    pc[:, :], lhsT=fast_sb["biasT8"][:, i * 128:(i + 1) * 128],
                    rhs=fast_sb["ones8b"][:, :], start=True, stop=False,
                )
                for pl in range(8):
                    p = 8 * i + pl
                    lhsT = w8_sb[p // 64][:, (p % 64) * 128:(p % 64) * 128 + 128]
                    nc.tensor.matmul(
                        pc[:, pl * 64:(pl + 1) * 64], lhsT=lhsT,
                        rhs=R[:, p * 64:(p + 1) * 64],
                        start=False, stop=True,
                    )
                evac(Rn[:, i * 512:(i + 1) * 512], pc[:, :], None, l8rot)
                if i % 2 == 1:
                    # out batch for V8 pairs of psums i-1, i: 8 two-pair
                    # matmuls, lhsT = contiguous 128-col R8 slice (pairs
                    # 2G, 2G+1), rhs = matching 64-col fea slice.  Valid
                    # psum quadrants: [0:64, 64g:64g+32] (pair 2G) and
                    # [64:128, 64g+32:64g+64] (pair 2G+1).
                    bt = i // 2
                    pco = po.tile([128, 512], F32, tag="po")
                    for g in range(8):
                        gg = 8 * bt + g      # global 2-pair group
                        nc.tensor.matmul(
                            pco[:, g * 64:(g + 1) * 64],
                            lhsT=Rn[:, gg * 128:(gg + 1) * 128],
                            rhs=fea_sb[:, gg * 64:(gg + 1) * 64],
                            start=True, stop=True,
                        )
                    o_sb = op.tile([BC, 512], F32, tag="os")
                    pcv = pco[:, :].rearrange("a (g c) -> a g c", c=64)
                    ov = o_sb[:, :].rearrange("a (g c) -> a g c", c=64)
                    if bt % 2 == 0:
                        nc.scalar.copy(ov[:, :, 0:32], pcv[0:64, :, 0:32])
                        nc.vector.tensor_copy(ov[:, :, 32:64], pcv[64:128, :, 32:64])
                    else:
                        nc.vector.tensor_copy(ov[:, :, 0:32], pcv[0:64, :, 0:32])
                        nc.scalar.copy(ov[:, :, 32:64], pcv[64:128, :, 32:64])
                    nc.sync.dma_start(out.ap()[:, bt * 512:(bt + 1) * 512],
                                      o_sb[:, :])
            R = Rn

    nc.compile()
    return nc


def _make_runner(nc):
    """Cached jitted SPMD runner over the 8 cores (mirrors
    bass2jax.run_bass_via_pjrt, but reusable across calls and without
    donation — this kernel writes every output element)."""
    import jax

    from concourse.bass2jax import (
        _bass_exec_p,
        install_neuronx_cc_hook,
        partition_id_tensor,
    )
    from jax.experimental.shard_map import shard_map
    from jax.sharding import Mesh, PartitionSpec

    install_neuronx_cc_hook()

    partition_name = nc.partition_id_tensor.name if nc.partition_id_tensor else None
    in_names: list[str] = []
    out_names: list[str] = []
    out_avals = []
    zero_outs: list[np.ndarray] = []
    for alloc in nc.m.functions[0].allocations:
        if not isinstance(alloc, mybir.MemoryLocationSet):
            continue
        name = alloc.memorylocations[0].name
        if alloc.kind == "ExternalInput":
            if name != partition_name:
                in_names.append(name)
        elif alloc.kind == "ExternalOutput":
            shape = tuple(alloc.tensor_shape)
            dtype = mybir.dt.np(alloc.dtype)
            out_names.append(name)
            out_avals.append(jax.core.ShapedArray(shape, dtype))
            zero_outs.append(np.zeros(shape, dtype))
    n_params = len(in_names)
    all_names = in_names + out_names
    if partition_name is not None:
        all_names = all_names + [partition_name]

    def _body(*args):
        operands = list(args)
        if partition_name is not None:
            operands.append(partition_id_tensor())
        outs = _bass_exec_p.bind(
            *operands,
            out_avals=tuple(out_avals),
            in_names=tuple(all_names),
            out_names=tuple(out_names),
            lowering_input_output_aliases=(),
            sim_require_finite=True,
            sim_require_nnan=True,
            nc=nc,
        )
        return tuple(outs)

    devices = jax.devices()[:NCORES]
    mesh = Mesh(np.asarray(devices), ("core",))
    n_all = n_params + len(out_names)
    sharded = jax.jit(
        shard_map(
            _body, mesh=mesh,
            in_specs=(PartitionSpec("core"),) * n_all,
            out_specs=(PartitionSpec("core"),) * len(out_names),
            check_rep=False,
        ),
        keep_unused=True,
    )
    return {
        "fn": sharded,
        "in_names": in_names,
        "out_names": out_names,
        "out_avals": out_avals,
        "zero_outs": zero_outs,
    }


def _runner():
    if "nc" not in _CACHE:
        _CACHE["nc"] = _build_module()
    if "runner" not in _CACHE:
        _CACHE["runner"] = _make_runner(_CACHE["nc"])
    return _CACHE["runner"]


def _concat_args(in_maps):
    r = _runner()
    args = [
        np.concatenate([np.asarray(m[name]) for m in in_maps], axis=0)
        for name in r["in_names"]
    ]
    args += [
        np.zeros((NCORES * z.shape[0], *z.shape[1:]), z.dtype) for z in r["zero_outs"]
    ]
    return args


def kernel(**inputs) -> np.ndarray:
    r = _runner()
    shared = pack_shared(
        inputs["W_in"], inputs["b_in"], inputs["W_lvl"], inputs["b_lvl"], inputs["Fea"]
    )
    in_data = np.asarray(inputs["in_data"], np.float32)
    in_maps = []
    for c in range(NCORES):
        m = dict(shared)
        m["xt"] = pack_x(in_data[c * BC:(c + 1) * BC])
        in_maps.append(m)

    out_arrs = r["fn"](*_concat_args(in_maps))
    out = np.asarray(out_arrs[r["out_names"].index("out")])
    return out.reshape(B, KTOT * FOUT, 1).astype(np.float32)
